# revision 7
# baseline (speedup 1.0000x reference)
"""Bass/Trainium2 kernel for nn_BerpXposMultiHeadedAttention (8-core SPMD).

Sharding: data-parallel over batch (4 batches x 2 cores) x tensor-parallel over
heads (4 heads per core).  Each core computes its 4 heads of flash-style xpos
attention for its batch plus the row-sharded partial out-projection; the host
sums the two partials per batch (the "all-reduce") and adds the output bias.

v2 redesign notes (per-core, per the instruction cost model):
- DMA count is the dominant fixed cost (~0.6us SP dispatch + ~0.6us shared
  HWDGE per DMA).  All weights ship in ONE packed [128, 7424] bf16 tensor,
  xpos tables in ONE [128, 4L] fp16 tensor, x tensors in 4 whole-L chunks
  each, and the output in 4 merged DMAs (custom strided AP).  144 DMAs -> ~23.
- The softmax denominator broadcast no longer round-trips DRAM: reciprocal
  of PSUM row 64 -> [1, TB] SBUF, then gpsimd partition_broadcast -> [64, TB],
  then one DVE mul writes normalized attn.T straight into attnT (engines can
  write a different partition base than they read - verified on HW).
- Causal masking moved off DVE/PSUM: scores are exp'd unmasked (diag-block
  scores are bounded, no overflow) and the bf16 probs multiplied by a 0/1
  lower-tri tile on the Pool engine (gpsimd = SBUF-only; it cannot touch
  PSUM, which rules out running PSUM ops there).
- exp activations and P@V matmuls are narrowed to the causally-live columns
  (above-diagonal 128-blocks skipped), removing all post-exp memsets.
- The xpos combine add (cos-term + sin-term) runs on Pool; the two PSUM-side
  muls stay on DVE.  Engine busy balance (model): PE ~100us, ACT ~70us,
  DVE ~50us, Pool ~45us, SP ~14us.
"""

import sys

sys.path.insert(0, "/opt/trn_rl_repo")

import contextlib

import numpy as np

import concourse.bacc as bacc
import concourse.bass as bass
import concourse.tile as tile
from concourse import mybir
from concourse.bass_utils import run_bass_kernel_spmd

# Problem constants (hardcoded per the task contract).
B = 4
L = 2048
EMBED = 512
HEADS = 8
HD = 64
SCALE_BASE = 512
NEG = -1e9
N_CORES = 8
HPC = 4           # heads per core
TB = 512          # t-block (strip) width
NS = L // 128     # 16 s-chunks
NSTRIP = L // TB  # 4 strips
VW = 328          # v_aug tile width (4 heads x 65 + 68 pad)

F32 = mybir.dt.float32
F32R = mybir.dt.float32r
F16 = mybir.dt.float16
BF16 = mybir.dt.bfloat16

# Deinterleave permutation of a 64-wide head dim: evens then odds.
_PERM64 = np.concatenate([np.arange(0, HD, 2), np.arange(1, HD, 2)])

# wpack column layout: 5 projections (qc,qs,kc,ks,v) x 4 chunks x 256 cols,
# then wo 2 chunks x 512 cols, then 5 bias rows x 256 cols (row 0 only).
_WNM = ["qc", "qs", "kc", "ks", "v"]
_WBASE = {nm: i * 1024 for i, nm in enumerate(_WNM)}
_WB_BASE = 5 * 1024
_WO_BASE = _WB_BASE + 5 * 256
_WCOLS = _WO_BASE + 2 * EMBED


def _xpos_tables():
    """Host-side xpos cos/sin tables in the permuted [d, t] layout.

    Returns (cq, sq, ck, sk), each [128, L] float32 (two heads' worth of rows,
    identical per head).  The 1/sqrt(HD) score scale is folded into the q pair.
    """
    d = HD
    base = ((np.arange(0, d, 2, dtype=np.float32) + np.float32(0.4 * d))
            / np.float32(1.4 * d)).astype(np.float32)                    # [32]
    min_pos = -(L // 2)
    power = (np.arange(min_pos, L + min_pos, dtype=np.float32)
             / np.float32(SCALE_BASE))                                   # [L]
    scale = (base[None, :] ** power[:, None]).astype(np.float32)         # [L, 32]
    half = d // 2
    inv_freq = (1.0 / (10000.0 ** (np.arange(half, dtype=np.float32) / half))
                ).astype(np.float32)
    sinusoid = np.arange(L, dtype=np.float32)[:, None] * inv_freq[None, :]
    sin = np.sin(sinusoid).astype(np.float32)
    cos = np.cos(sinusoid).astype(np.float32)

    def pack(cs, ss, fold):
        cs = (cs * fold).astype(np.float32)
        ss = (ss * fold).astype(np.float32)
        # permuted layout: rows 0:32 <- even orig dims, rows 32:64 <- odd.
        cos_p = np.concatenate([cs.T, cs.T], axis=0)      # [64, L]
        sin_p = np.concatenate([-ss.T, ss.T], axis=0)     # [64, L]
        return (np.concatenate([cos_p, cos_p], axis=0).astype(np.float32),
                np.concatenate([sin_p, sin_p], axis=0).astype(np.float32))

    inv_scale = (1.0 / scale).astype(np.float32)
    cq, sq = pack(cos * scale, sin * scale, np.float32(HD ** -0.5))
    ck, sk = pack(cos * inv_scale, sin * inv_scale, np.float32(1.0))
    return cq, sq, ck, sk


def _build_program(causal: bool, use_mask: bool, has_bias: bool, reps: int = 1):
    nc = bacc.Bacc("TRN2", target_bir_lowering=False, debug=False,
                   num_devices=N_CORES)

    # ---- DRAM I/O -------------------------------------------------------
    xqT = nc.dram_tensor("xqT", [EMBED, L], F16, kind="ExternalInput")
    xkT = nc.dram_tensor("xkT", [EMBED, L], F16, kind="ExternalInput")
    xvT = nc.dram_tensor("xvT", [EMBED, L], F16, kind="ExternalInput")
    wpackD = nc.dram_tensor("wpack", [128, _WCOLS], BF16, kind="ExternalInput")
    tabsD = nc.dram_tensor("tabs", [128, 4 * L], F16, kind="ExternalInput")
    triD = nc.dram_tensor("tri01", [128, 128], BF16, kind="ExternalInput")
    maskD = None
    if use_mask:
        maskD = nc.dram_tensor("maskT", [L, L], F32, kind="ExternalInput")
    outp = nc.dram_tensor("outp", [L, EMBED], F16, kind="ExternalOutput")

    xin = {"q": xqT, "k": xkT, "v": xvT}
    # tab column base per table name
    tb_base = {"cq": 0, "sq": L, "ck": 2 * L, "sk": 3 * L}

    with tile.TileContext(nc) as tc:
        with contextlib.ExitStack() as ctx:
            consts = ctx.enter_context(tc.tile_pool(name="consts", bufs=1))
            xpool = ctx.enter_context(tc.tile_pool(name="xpool", bufs=12))
            qkpool = ctx.enter_context(tc.tile_pool(name="qkpool", bufs=1))
            vpool = ctx.enter_context(tc.tile_pool(name="vpool", bufs=NS))
            tmp = ctx.enter_context(tc.tile_pool(name="tmp", bufs=4))
            ptpool = ctx.enter_context(tc.tile_pool(name="ptpool", bufs=10))
            npool = ctx.enter_context(tc.tile_pool(name="npool", bufs=4))
            bcpool = ctx.enter_context(tc.tile_pool(name="bcpool", bufs=4))
            opool = ctx.enter_context(tc.tile_pool(name="opool", bufs=2))
            mpool = None
            if use_mask:
                mpool = ctx.enter_context(tc.tile_pool(name="mpool", bufs=NS + 2))
            ps_s = ctx.enter_context(tc.tile_pool(name="ps_s", bufs=2, space="PSUM"))
            ps_pv = ctx.enter_context(tc.tile_pool(name="ps_pv", bufs=4, space="PSUM"))

            def body():
                # ---- constants / weights: 3 DMAs total ----
                wsb = consts.tile([128, _WO_BASE], BF16, tag="wsb", name="wsb")
                nc.sync.dma_start(wsb[:], wpackD[:, 0:_WO_BASE])
                wosb = consts.tile([128, 2 * EMBED], BF16, tag="wosb",
                                   name="wosb")
                nc.sync.dma_start(wosb[:], wpackD[:, _WO_BASE:_WCOLS])
                tabsb = consts.tile([128, 4 * L], F16, tag="tabs", name="tabs")
                nc.sync.dma_start(tabsb[:], tabsD[:])
                tri_sb = consts.tile([128, 128], BF16, tag="tri")
                if causal:
                    nc.sync.dma_start(tri_sb[:], triD[:])
                ones_sb = None
                if has_bias:
                    ones_sb = consts.tile([1, L], F16, tag="ones")
                    nc.vector.memset(ones_sb[:], 1.0)

                attnT = [consts.tile([128, L], BF16, tag=f"attnT{c}",
                                     name=f"attnT{c}") for c in range(2)]

                # ---- x loads: 12 DMAs of [128, L] ----
                xsb = {}
                for nm in ("q", "k", "v"):
                    chunks = []
                    for c in range(4):
                        t = xpool.tile([128, L], F16, tag="x",
                                       name=f"x{nm}{c}")
                        nc.sync.dma_start(t[:], xin[nm][c * 128:(c + 1) * 128, :])
                        chunks.append(t)
                    xsb[nm] = chunks

                qTt = [[None] * NSTRIP for _ in range(2)]  # [e][tb]
                kTt = [[None] * NSTRIP for _ in range(2)]
                vaug = [None] * NS

                def proj_qk(nm, tb, dst):
                    ts = slice(tb * TB, (tb + 1) * TB)
                    cb = tb_base[{"q": "cq", "k": "ck"}[nm]]
                    sb_ = tb_base[{"q": "sq", "k": "sk"}[nm]]
                    for e in range(2):
                        ps = ps_s.tile([128, 1024], F32, tag="s",
                                       name=f"ps_{nm}{e}_{tb}")
                        for half, wkey in ((0, nm + "c"), (1, nm + "s")):
                            wb = _WBASE[wkey]
                            for c in range(4):
                                nc.tensor.matmul(
                                    ps[:, half * TB:(half + 1) * TB],
                                    wsb[:, wb + c * 256 + e * 128:
                                        wb + c * 256 + e * 128 + 128],
                                    xsb[nm][c][:, ts],
                                    start=(c == 0),
                                    stop=(c == 3 and not has_bias))
                            if has_bias:
                                bi = _WNM.index(wkey)
                                nc.tensor.matmul(
                                    ps[:, half * TB:(half + 1) * TB],
                                    wsb[0:1, _WB_BASE + bi * 256 + e * 128:
                                        _WB_BASE + bi * 256 + e * 128 + 128],
                                    ones_sb[:, ts], start=False, stop=True)
                        t1 = tmp.tile([128, TB], F32, tag="t1",
                                      name=f"t1{nm}{e}{tb}")
                        nc.vector.tensor_mul(t1[:], ps[:, 0:TB],
                                             tabsb[:, cb + tb * TB:cb + (tb + 1) * TB])
                        t2 = tmp.tile([128, TB], F32, tag="t2",
                                      name=f"t2{nm}{e}{tb}")
                        nc.vector.tensor_mul(t2[:], ps[:, TB:1024],
                                             tabsb[:, sb_ + tb * TB:sb_ + (tb + 1) * TB])
                        ot = qkpool.tile([128, TB], F32R, tag=f"{nm}T{e}_{tb}",
                                         name=f"{nm}T{e}_{tb}")
                        with nc.allow_low_precision(reason="f32r is fp32 bits"):
                            nc.vector.tensor_add(ot[:], t1[:], t2[:])
                        dst[e][tb] = ot

                def proj_v(tb):
                    wb = _WBASE["v"]
                    for j in range(4):
                        si = tb * 4 + j
                        js = slice(si * 128, (si + 1) * 128)
                        ps = ps_pv.tile([128, 256], F32, tag="pv",
                                        name=f"ps_v{si}")
                        for c in range(4):
                            nc.tensor.matmul(ps[:], xsb["v"][c][:, js],
                                             wsb[:, wb + c * 256:wb + (c + 1) * 256],
                                             start=(c == 0),
                                             stop=(c == 3 and not has_bias))
                        if has_bias:
                            nc.tensor.matmul(
                                ps[:], ones_sb[:, js],
                                wsb[0:1, _WB_BASE + 4 * 256:_WB_BASE + 5 * 256],
                                start=False, stop=True)
                        va = vpool.tile([128, VW], BF16, tag="vaug",
                                        name=f"vaug{si}")
                        va3 = va[:, 0:HPC * 65].rearrange("p (h c) -> p h c", c=65)
                        nc.vector.tensor_copy(
                            va3[:, :, 0:64],
                            ps[:].rearrange("p (h d) -> p h d", d=64))
                        nc.vector.memset(va3[:, :, 64:65], 1.0)
                        nc.vector.memset(va[:, HPC * 65:VW], 0.0)
                        vaug[si] = va

                def flash_strip(T):
                    nsig = 4 * T + 4 if causal else NS
                    mtiles = None
                    if use_mask:
                        mtiles = []
                        for si in range(nsig):
                            mt = mpool.tile([128, TB], F32, tag="mask",
                                            name=f"m{T}_{si}")
                            nc.sync.dma_start(
                                mt[:], maskD[si * 128:(si + 1) * 128,
                                             T * TB:(T + 1) * TB])
                            mtiles.append(mt)
                    tcols = slice(T * TB, (T + 1) * TB)
                    for ht in range(2):
                        # Both heads of the pair advance together: their QK
                        # matmuls contract over disjoint 64-partition row
                        # groups (base 0 / 64) and are issued back-to-back so
                        # the PE array runs them concurrently (K=64 would
                        # otherwise idle half the rows).
                        po = [ps_pv.tile([128, TB], F32, tag="pv",
                                         name=f"po{T}t{ht}e{e}")
                              for e in range(2)]
                        for sig in range(nsig):
                            j = sig - 4 * T
                            jc = max(j, 0) if causal else 0
                            coff = 0
                            if causal and j >= 0:
                                coff = TB - max(TB - j * 128, 256)
                            ps2 = ps_s.tile([128, 1024], F32, tag="s",
                                            name=f"S{T}t{ht}s{sig}")
                            for e in range(2):
                                hr = e * 64
                                nc.tensor.matmul(
                                    ps2[:, e * TB + coff:(e + 1) * TB],
                                    kTt[ht][sig // 4][hr:hr + 64,
                                                      (sig % 4) * 128:
                                                      (sig % 4 + 1) * 128],
                                    qTt[ht][T][hr:hr + 64, coff:TB],
                                    start=True, stop=True)
                            if use_mask:
                                for e in range(2):
                                    sl = slice(e * TB, (e + 1) * TB)
                                    nc.vector.tensor_add(ps2[:, sl], ps2[:, sl],
                                                         mtiles[sig][:])
                            pt = ptpool.tile([128, 1024], BF16, tag="pt",
                                             name=f"P{T}t{ht}s{sig}")
                            if jc > 0:
                                pt3 = pt.rearrange("p (h t) -> p h t", t=TB)
                                ps3 = ps2.rearrange("p (h t) -> p h t", t=TB)
                                nc.scalar.activation(
                                    pt3[:, :, jc * 128:TB],
                                    ps3[:, :, jc * 128:TB],
                                    mybir.ActivationFunctionType.Exp)
                            else:
                                nc.scalar.activation(
                                    pt[:], ps2[:],
                                    mybir.ActivationFunctionType.Exp)
                            if causal and 0 <= j <= 3:
                                for e in range(2):
                                    sl = slice(e * TB + j * 128,
                                               e * TB + (j + 1) * 128)
                                    with nc.allow_low_precision(
                                            reason="bf16 prob masking"):
                                        nc.vector.tensor_mul(
                                            pt[:, sl], pt[:, sl], tri_sb[:])
                            for e in range(2):
                                nc.tensor.matmul(
                                    po[e][:, jc * 128:TB],
                                    vaug[sig][:, (2 * ht + e) * 65:
                                              (2 * ht + e) * 65 + 128],
                                    pt[:, e * TB + jc * 128:(e + 1) * TB],
                                    start=(sig == 0), stop=(sig == nsig - 1))
                        # normalization: reciprocal of the denominator row,
                        # partition-broadcast it (Pool), one DVE mul writes
                        # normalized attn.T into attnT (partition-shifted for
                        # odd heads).
                        for e in range(2):
                            rec = npool.tile([1, TB], F32, tag="rec",
                                             name=f"rec{T}t{ht}e{e}")
                            nc.vector.reciprocal(rec[:], po[e][64:65, :])
                            bc = bcpool.tile([64, TB], F32, tag="bc",
                                             name=f"bc{T}t{ht}e{e}")
                            nc.gpsimd.partition_broadcast(bc[:], rec[:])
                            with nc.allow_low_precision(reason="bf16 attn out"):
                                nc.vector.tensor_mul(
                                    attnT[ht][e * 64:e * 64 + 64, tcols],
                                    po[e][0:64, :], bc[:])

                def out_proj():
                    for go in range(4):
                        osb = opool.tile([128, 4 * EMBED], F16, tag="osb",
                                         name=f"osb{go}")
                        for ti in range(4):
                            tau = go * 4 + ti
                            ps = ps_s.tile([128, EMBED], F32, tag="s",
                                           name=f"ps_o{tau}")
                            for c in range(2):
                                nc.tensor.matmul(
                                    ps[:], attnT[c][:, tau * 128:(tau + 1) * 128],
                                    wosb[:, c * EMBED:(c + 1) * EMBED],
                                    start=(c == 0), stop=(c == 1))
                            with nc.allow_low_precision(reason="fp16 partial out"):
                                nc.any.tensor_copy(
                                    osb[:, ti * EMBED:(ti + 1) * EMBED], ps[:])
                        dst = bass.AP(
                            tensor=outp[:].tensor,
                            offset=go * 4 * 128 * EMBED,
                            ap=[[EMBED, 128], [128 * EMBED, 4], [1, EMBED]])
                        nc.sync.dma_start(dst, osb[:])

                for tb in range(NSTRIP):
                    proj_qk("q", tb, qTt)
                    proj_qk("k", tb, kTt)
                    proj_v(tb)
                    flash_strip(tb)
                out_proj()

            if reps > 1:
                with tc.For_i(0, reps, 1,
                              hint_engines=(mybir.EngineType.PE,
                                            mybir.EngineType.Activation,
                                            mybir.EngineType.DVE,
                                            mybir.EngineType.SP,
                                            mybir.EngineType.Pool)):
                    body()
            else:
                body()

    nc.compile()
    return nc


_PROGRAM_CACHE = {}


def get_program(causal: bool, use_mask: bool, has_bias: bool, reps: int = 1):
    key = (causal, use_mask, has_bias, reps)
    if key not in _PROGRAM_CACHE:
        _PROGRAM_CACHE[key] = _build_program(causal, use_mask, has_bias, reps)
    return _PROGRAM_CACHE[key]


def _prep_in_maps(query, key, value, key_padding_mask, attn_mask,
                  Wq, bq, Wk, bk, Wv, bv, Wo, bo, use_mask, has_bias):
    """Build the 8 per-core input dicts."""
    import ml_dtypes
    cq, sq, ck, sk = _xpos_tables()
    tabs = np.concatenate([cq, sq, ck, sk], axis=1).astype(np.float16)
    # tri01[s, t] = 1 where t >= s (keep), else 0  (probs layout [s, t])
    tri01 = np.where(np.arange(128)[None, :] >= np.arange(128)[:, None],
                     np.float32(1.0), np.float32(0.0)).astype(ml_dtypes.bfloat16)

    def xT(x):
        return np.ascontiguousarray(
            np.asarray(x, np.float32).T.astype(np.float16))

    xqTs = [xT(query[b]) for b in range(B)]
    xkTs = [xT(key[b]) for b in range(B)]
    xvTs = [xT(value[b]) for b in range(B)]

    masks = None
    if use_mask:
        am = np.asarray(attn_mask, np.float32)
        kp = np.asarray(key_padding_mask)
        masks = []
        for b in range(B):
            m = am.copy()
            if kp[b].any():
                m = m + np.where(kp[b], np.float32(-1e30),
                                 np.float32(0.0))[None, :]
            masks.append(np.ascontiguousarray(m.T.astype(np.float32)))

    Wq = np.asarray(Wq, np.float32); bq = np.asarray(bq, np.float32)
    Wk = np.asarray(Wk, np.float32); bk = np.asarray(bk, np.float32)
    Wv = np.asarray(Wv, np.float32); bv = np.asarray(bv, np.float32)
    Wo = np.asarray(Wo, np.float32)

    in_maps = []
    for core in range(N_CORES):
        b, hg = core // 2, core % 2
        hs = hg * HPC
        idx_p = np.concatenate(
            [hs * HD + hl * HD + _PERM64 for hl in range(HPC)])
        # sin-projection rows: within each head's 64-block, row r <- r XOR 32
        xor = (np.arange(256).reshape(HPC, HD)[:, (np.arange(HD) ^ 32)]
               ).reshape(-1)
        idx_s = idx_p[xor]
        idx_v = hs * HD + np.arange(HPC * HD)

        wpack = np.zeros((128, _WCOLS), np.float32)
        wsel = {"qc": (Wq, bq, idx_p), "qs": (Wq, bq, idx_s),
                "kc": (Wk, bk, idx_p), "ks": (Wk, bk, idx_s),
                "v": (Wv, bv, idx_v)}
        for i, nm in enumerate(_WNM):
            W, bias, idx = wsel[nm]
            WT = W[idx, :].T          # [512 embed-in, 256 sel dims]
            for c in range(4):
                wpack[:, i * 1024 + c * 256:(i * 1024) + (c + 1) * 256] = \
                    WT[c * 128:(c + 1) * 128, :]
            if has_bias:
                wpack[0, _WB_BASE + i * 256:_WB_BASE + (i + 1) * 256] = bias[idx]
        WoT = Wo[:, idx_v].T          # [256 attn dims, 512 embed-out]
        wpack[:, _WO_BASE:_WO_BASE + EMBED] = WoT[0:128, :]
        wpack[:, _WO_BASE + EMBED:_WO_BASE + 2 * EMBED] = WoT[128:256, :]

        m = {
            "xqT": xqTs[b], "xkT": xkTs[b], "xvT": xvTs[b],
            "wpack": wpack.astype(ml_dtypes.bfloat16),
            "tabs": tabs, "tri01": tri01,
        }
        if use_mask:
            m["maskT"] = masks[b]
        in_maps.append(m)
    return in_maps


def classify_mask(attn_mask, key_padding_mask):
    am = np.asarray(attn_mask, np.float32)
    kp = np.asarray(key_padding_mask)
    if not kp.any():
        causal = np.where(
            np.tril(np.ones((L, L), bool)), np.float32(0.0),
            np.float32(NEG)).astype(np.float32)
        if np.array_equal(am, causal):
            return True, False
        if not am.any():
            return False, False
    return False, True


def kernel(query, key, value, key_padding_mask, attn_mask,
           Wq, bq, Wk, bk, Wv, bv, Wo, bo):
    causal, use_mask = classify_mask(attn_mask, key_padding_mask)
    has_bias = bool(np.asarray(bq).any() or np.asarray(bk).any()
                    or np.asarray(bv).any())
    nc = get_program(causal, use_mask, has_bias, reps=1)
    in_maps = _prep_in_maps(query, key, value, key_padding_mask, attn_mask,
                            Wq, bq, Wk, bk, Wv, bv, Wo, bo, use_mask, has_bias)
    res = run_bass_kernel_spmd(nc, in_maps, list(range(N_CORES)))
    bo = np.asarray(bo, np.float32)
    out = np.empty((B, L, EMBED), np.float32)
    for b in range(B):
        out[b] = (res.results[2 * b]["outp"].astype(np.float32)
                  + res.results[2 * b + 1]["outp"].astype(np.float32)
                  + bo[None, :])
    return out


# revision 9
# speedup vs baseline: 1.2848x; 1.2848x over previous
"""Bass/Trainium2 kernel for nn_BerpXposMultiHeadedAttention (8-core SPMD).

Sharding: data-parallel over batch (4 batches x 2 cores) x tensor-parallel over
heads (4 heads per core).  Each core computes its 4 heads of flash-style xpos
attention for its batch plus the row-sharded partial out-projection; the host
sums the two partials per batch (the "all-reduce") and adds the output bias.

v2 design notes (HW-measured on trn2; 343us/rep baseline -> ~245us/rep):
- DMA count dominates fixed cost (~0.6us SP dispatch + ~0.6us shared HWDGE
  per DMA).  All projection weights ship in ONE packed [128, 6400] bf16
  tensor + wo in a second tile (separate tile so the rep-boundary reload
  does not serialize behind out_proj), xpos tables in ONE [128, 4L] fp16
  tensor, x in 4 whole-L chunks per tensor, output in 4 merged DMAs with a
  custom strided AP.  144 DMAs -> ~21.
- Softmax denominator broadcast without the DRAM round-trip: DVE reciprocal
  of PSUM row 64 -> [1, TB] SBUF, gpsimd partition_broadcast -> [64, TB],
  one DVE mul writes normalized attn.T into attnT, using a partition-shifted
  engine write for odd heads (verified legal on HW; stride-0 SBUF DMA src is
  not, which is why the old kernel bounced through DRAM).
- gpsimd (Pool) elementwise is a TRAP: moving the xpos adds / prob masking /
  memsets there regressed HW by 140us despite the cost model approving
  (software Q7 ops + per-op library reloads).  Only partition_broadcast
  (which has no other-engine equivalent) stays on Pool.
- Causal masking costs no PSUM ops: scores are exp'd unmasked (diag-block
  scores are bounded so exp cannot overflow) and the bf16 probs are
  multiplied by a 0/1 lower-tri tile on DVE (2-byte dtypes get DVE 2x mode).
- exp activations and P@V matmuls are narrowed to causally-live columns;
  no post-exp memsets.  The two heads sharing a qT/kT tile are processed in
  lockstep per s-block: their K=64 QK matmuls contract over partition row
  groups 0/64 and are issued back-to-back into different PSUM banks of one
  [128,1024] tile, giving one [128, 2x512] exp per s-block (80 activations
  instead of 124).
- QK^T stays fp32r x fp32r: HW-benched LDW+MM at K=64,N=512 is ~310ns for
  f32r vs ~380ns for fp16/bf16 (and exact), contrary to the cost model's
  213ns-for-everything.  Per-MM cost is LDWEIGHTS-exposed; 16-bit dtypes do
  not help matmul streams whose stationary changes every MM.
"""
import os
import sys

sys.path.insert(0, "/opt/trn_rl_repo")

_ABL = set(os.environ.get("ABL", "").split(","))

import contextlib

import numpy as np

import concourse.bacc as bacc
import concourse.bass as bass
import concourse.tile as tile
from concourse import mybir
from concourse.bass_utils import run_bass_kernel_spmd

# Problem constants (hardcoded per the task contract).
B = 4
L = 2048
EMBED = 512
HEADS = 8
HD = 64
SCALE_BASE = 512
NEG = -1e9
N_CORES = 8
HPC = 4           # heads per core
TB = 512          # t-block (strip) width
NS = L // 128     # 16 s-chunks
NSTRIP = L // TB  # 4 strips
VW = 328          # v_aug tile width (4 heads x 65 + 68 pad)

F32 = mybir.dt.float32
F32R = mybir.dt.float32r
F16 = mybir.dt.float16
BF16 = mybir.dt.bfloat16

# Deinterleave permutation of a 64-wide head dim: evens then odds.
_PERM64 = np.concatenate([np.arange(0, HD, 2), np.arange(1, HD, 2)])

# wpack column layout: 5 projections (qc,qs,kc,ks,v) x 4 chunks x 256 cols,
# then wo 2 chunks x 512 cols, then 5 bias rows x 256 cols (row 0 only).
_WNM = ["qc", "qs", "kc", "ks", "v"]
_WBASE = {nm: i * 1024 for i, nm in enumerate(_WNM)}
_WB_BASE = 5 * 1024
_WO_BASE = _WB_BASE + 5 * 256
_WCOLS = _WO_BASE + 2 * EMBED


def _xpos_tables():
    """Host-side xpos cos/sin tables in the permuted [d, t] layout.

    Returns (cq, sq, ck, sk), each [128, L] float32 (two heads' worth of rows,
    identical per head).  The 1/sqrt(HD) score scale is folded into the q pair.
    """
    d = HD
    base = ((np.arange(0, d, 2, dtype=np.float32) + np.float32(0.4 * d))
            / np.float32(1.4 * d)).astype(np.float32)                    # [32]
    min_pos = -(L // 2)
    power = (np.arange(min_pos, L + min_pos, dtype=np.float32)
             / np.float32(SCALE_BASE))                                   # [L]
    scale = (base[None, :] ** power[:, None]).astype(np.float32)         # [L, 32]
    half = d // 2
    inv_freq = (1.0 / (10000.0 ** (np.arange(half, dtype=np.float32) / half))
                ).astype(np.float32)
    sinusoid = np.arange(L, dtype=np.float32)[:, None] * inv_freq[None, :]
    sin = np.sin(sinusoid).astype(np.float32)
    cos = np.cos(sinusoid).astype(np.float32)

    def pack(cs, ss, fold):
        cs = (cs * fold).astype(np.float32)
        ss = (ss * fold).astype(np.float32)
        # permuted layout: rows 0:32 <- even orig dims, rows 32:64 <- odd.
        cos_p = np.concatenate([cs.T, cs.T], axis=0)      # [64, L]
        sin_p = np.concatenate([-ss.T, ss.T], axis=0)     # [64, L]
        return (np.concatenate([cos_p, cos_p], axis=0).astype(np.float32),
                np.concatenate([sin_p, sin_p], axis=0).astype(np.float32))

    inv_scale = (1.0 / scale).astype(np.float32)
    cq, sq = pack(cos * scale, sin * scale, np.float32(HD ** -0.5))
    ck, sk = pack(cos * inv_scale, sin * inv_scale, np.float32(1.0))
    return cq, sq, ck, sk


def _build_program(causal: bool, use_mask: bool, has_bias: bool, reps: int = 1):
    nc = bacc.Bacc("TRN2", target_bir_lowering=False, debug=False,
                   num_devices=N_CORES)

    # ---- DRAM I/O -------------------------------------------------------
    xqT = nc.dram_tensor("xqT", [EMBED, L], F16, kind="ExternalInput")
    xkT = nc.dram_tensor("xkT", [EMBED, L], F16, kind="ExternalInput")
    xvT = nc.dram_tensor("xvT", [EMBED, L], F16, kind="ExternalInput")
    wpackD = nc.dram_tensor("wpack", [128, _WCOLS], BF16, kind="ExternalInput")
    tabsD = nc.dram_tensor("tabs", [128, 4 * L], F16, kind="ExternalInput")
    triD = nc.dram_tensor("tri01", [128, 128], BF16, kind="ExternalInput")
    maskD = None
    if use_mask:
        maskD = nc.dram_tensor("maskT", [L, L], F32, kind="ExternalInput")
    outp = nc.dram_tensor("outp", [L, EMBED], F16, kind="ExternalOutput")

    xin = {"q": xqT, "k": xkT, "v": xvT}
    # tab column base per table name
    tb_base = {"cq": 0, "sq": L, "ck": 2 * L, "sk": 3 * L}

    with tile.TileContext(nc) as tc:
        with contextlib.ExitStack() as ctx:
            consts = ctx.enter_context(tc.tile_pool(name="consts", bufs=1))
            xpool = ctx.enter_context(tc.tile_pool(name="xpool", bufs=12))
            qkpool = ctx.enter_context(tc.tile_pool(name="qkpool", bufs=1))
            vpool = ctx.enter_context(tc.tile_pool(name="vpool", bufs=NS))
            tmp = ctx.enter_context(tc.tile_pool(name="tmp", bufs=4))
            ptpool = ctx.enter_context(tc.tile_pool(name="ptpool", bufs=10))
            npool = ctx.enter_context(tc.tile_pool(name="npool", bufs=4))
            bcpool = ctx.enter_context(tc.tile_pool(name="bcpool", bufs=4))
            opool = ctx.enter_context(tc.tile_pool(name="opool", bufs=2))
            mpool = None
            if use_mask:
                mpool = ctx.enter_context(tc.tile_pool(name="mpool", bufs=NS + 2))
            ps_s = ctx.enter_context(tc.tile_pool(name="ps_s", bufs=2, space="PSUM"))
            ps_pv = ctx.enter_context(tc.tile_pool(name="ps_pv", bufs=4, space="PSUM"))

            def body():
                # ---- constants / weights: 3 DMAs total ----
                wsb = consts.tile([128, _WO_BASE], BF16, tag="wsb", name="wsb")
                nc.sync.dma_start(wsb[:], wpackD[:, 0:_WO_BASE])
                wosb = consts.tile([128, 2 * EMBED], BF16, tag="wosb",
                                   name="wosb")
                nc.sync.dma_start(wosb[:], wpackD[:, _WO_BASE:_WCOLS])
                tabsb = consts.tile([128, 4 * L], F16, tag="tabs", name="tabs")
                nc.sync.dma_start(tabsb[:], tabsD[:])
                tri_sb = consts.tile([128, 128], BF16, tag="tri")
                if causal:
                    nc.sync.dma_start(tri_sb[:], triD[:])
                ones_sb = None
                if has_bias:
                    ones_sb = consts.tile([1, L], F16, tag="ones")
                    nc.vector.memset(ones_sb[:], 1.0)

                attnT = [consts.tile([128, L], BF16, tag=f"attnT{c}",
                                     name=f"attnT{c}") for c in range(2)]

                # ---- x loads: 12 DMAs of [128, L] ----
                xsb = {}
                for nm in ("q", "k", "v"):
                    chunks = []
                    for c in range(4):
                        t = xpool.tile([128, L], F16, tag="x",
                                       name=f"x{nm}{c}")
                        nc.sync.dma_start(t[:], xin[nm][c * 128:(c + 1) * 128, :])
                        chunks.append(t)
                    xsb[nm] = chunks

                qTt = [[None] * NSTRIP for _ in range(2)]  # [e][tb]
                kTt = [[None] * NSTRIP for _ in range(2)]
                vaug = [None] * NS

                def proj_qk(nm, tb, dst):
                    ts = slice(tb * TB, (tb + 1) * TB)
                    cb = tb_base[{"q": "cq", "k": "ck"}[nm]]
                    sb_ = tb_base[{"q": "sq", "k": "sk"}[nm]]
                    for e in range(2):
                        if "noproj" in _ABL:
                            ot = qkpool.tile([128, TB], F32R, tag=f"{nm}T{e}_{tb}",
                                             name=f"{nm}T{e}_{tb}")
                            dst[e][tb] = ot
                            continue
                        ps = ps_s.tile([128, 1024], F32, tag="s",
                                       name=f"ps_{nm}{e}_{tb}")
                        for half, wkey in ((0, nm + "c"), (1, nm + "s")):
                            wb = _WBASE[wkey]
                            for c in range(4):
                                nc.tensor.matmul(
                                    ps[:, half * TB:(half + 1) * TB],
                                    wsb[:, wb + c * 256 + e * 128:
                                        wb + c * 256 + e * 128 + 128],
                                    xsb[nm][c][:, ts],
                                    start=(c == 0),
                                    stop=(c == 3 and not has_bias))
                            if has_bias:
                                bi = _WNM.index(wkey)
                                nc.tensor.matmul(
                                    ps[:, half * TB:(half + 1) * TB],
                                    wsb[0:1, _WB_BASE + bi * 256 + e * 128:
                                        _WB_BASE + bi * 256 + e * 128 + 128],
                                    ones_sb[:, ts], start=False, stop=True)
                        t1 = tmp.tile([128, TB], F32, tag="t1",
                                      name=f"t1{nm}{e}{tb}")
                        nc.vector.tensor_mul(t1[:], ps[:, 0:TB],
                                             tabsb[:, cb + tb * TB:cb + (tb + 1) * TB])
                        t2 = tmp.tile([128, TB], F32, tag="t2",
                                      name=f"t2{nm}{e}{tb}")
                        nc.vector.tensor_mul(t2[:], ps[:, TB:1024],
                                             tabsb[:, sb_ + tb * TB:sb_ + (tb + 1) * TB])
                        ot = qkpool.tile([128, TB], F32R, tag=f"{nm}T{e}_{tb}",
                                         name=f"{nm}T{e}_{tb}")
                        with nc.allow_low_precision(reason="f32r is fp32 bits"):
                            nc.vector.tensor_add(ot[:], t1[:], t2[:])
                        dst[e][tb] = ot

                def proj_v(tb):
                    wb = _WBASE["v"]
                    for j in range(4):
                        si = tb * 4 + j
                        js = slice(si * 128, (si + 1) * 128)
                        ps = ps_pv.tile([128, 256], F32, tag="pv",
                                        name=f"ps_v{si}")
                        for c in range(4):
                            nc.tensor.matmul(ps[:], xsb["v"][c][:, js],
                                             wsb[:, wb + c * 256:wb + (c + 1) * 256],
                                             start=(c == 0),
                                             stop=(c == 3 and not has_bias))
                        if has_bias:
                            nc.tensor.matmul(
                                ps[:], ones_sb[:, js],
                                wsb[0:1, _WB_BASE + 4 * 256:_WB_BASE + 5 * 256],
                                start=False, stop=True)
                        va = vpool.tile([128, VW], BF16, tag="vaug",
                                        name=f"vaug{si}")
                        va3 = va[:, 0:HPC * 65].rearrange("p (h c) -> p h c", c=65)
                        nc.vector.tensor_copy(
                            va3[:, :, 0:64],
                            ps[:].rearrange("p (h d) -> p h d", d=64))
                        nc.vector.memset(va3[:, :, 64:65], 1.0)
                        nc.vector.memset(va[:, HPC * 65:VW], 0.0)
                        vaug[si] = va

                def flash_strip(T):
                    nsig = 4 * T + 4 if causal else NS
                    mtiles = None
                    if use_mask:
                        mtiles = []
                        for si in range(nsig):
                            mt = mpool.tile([128, TB], F32, tag="mask",
                                            name=f"m{T}_{si}")
                            nc.sync.dma_start(
                                mt[:], maskD[si * 128:(si + 1) * 128,
                                             T * TB:(T + 1) * TB])
                            mtiles.append(mt)
                    tcols = slice(T * TB, (T + 1) * TB)
                    for ht in range(2):
                        # Both heads of the pair advance together: their QK
                        # matmuls contract over disjoint 64-partition row
                        # groups (base 0 / 64) and are issued back-to-back so
                        # the PE array runs them concurrently (K=64 would
                        # otherwise idle half the rows).
                        po = [ps_pv.tile([128, TB], F32, tag="pv",
                                         name=f"po{T}t{ht}e{e}")
                              for e in range(2)]
                        for sig in range(nsig):
                            j = sig - 4 * T
                            jc = max(j, 0) if causal else 0
                            coff = 0
                            if causal and j >= 0:
                                coff = TB - max(TB - j * 128, 256)
                            ps2 = ps_s.tile([128, 1024], F32, tag="s",
                                            name=f"S{T}t{ht}s{sig}")
                            for e in range(2) if "noqk" not in _ABL else []:
                                hr = e * 64
                                nc.tensor.matmul(
                                    ps2[:, e * TB + coff:(e + 1) * TB],
                                    kTt[ht][sig // 4][hr:hr + 64,
                                                      (sig % 4) * 128:
                                                      (sig % 4 + 1) * 128],
                                    qTt[ht][T][hr:hr + 64, coff:TB],
                                    start=True, stop=True)
                            if use_mask:
                                for e in range(2):
                                    sl = slice(e * TB, (e + 1) * TB)
                                    nc.vector.tensor_add(ps2[:, sl], ps2[:, sl],
                                                         mtiles[sig][:])
                            pt = ptpool.tile([128, 1024], BF16, tag="pt",
                                             name=f"P{T}t{ht}s{sig}")
                            if "noexp" in _ABL:
                                pass
                            elif jc > 0:
                                pt3 = pt.rearrange("p (h t) -> p h t", t=TB)
                                ps3 = ps2.rearrange("p (h t) -> p h t", t=TB)
                                nc.scalar.activation(
                                    pt3[:, :, jc * 128:TB],
                                    ps3[:, :, jc * 128:TB],
                                    mybir.ActivationFunctionType.Exp)
                            else:
                                nc.scalar.activation(
                                    pt[:], ps2[:],
                                    mybir.ActivationFunctionType.Exp)
                            if causal and 0 <= j <= 3:
                                for e in range(2):
                                    sl = slice(e * TB + j * 128,
                                               e * TB + (j + 1) * 128)
                                    with nc.allow_low_precision(
                                            reason="bf16 prob masking"):
                                        nc.vector.tensor_mul(
                                            pt[:, sl], pt[:, sl], tri_sb[:])
                            for e in range(2) if "nopv" not in _ABL else []:
                                nc.tensor.matmul(
                                    po[e][:, jc * 128:TB],
                                    vaug[sig][:, (2 * ht + e) * 65:
                                              (2 * ht + e) * 65 + 128],
                                    pt[:, e * TB + jc * 128:(e + 1) * TB],
                                    start=(sig == 0), stop=(sig == nsig - 1))
                        # normalization: reciprocal of the denominator row,
                        # partition-broadcast it (Pool), one DVE mul writes
                        # normalized attn.T into attnT (partition-shifted for
                        # odd heads).
                        for e in range(2) if "nonorm" not in _ABL else []:
                            rec = npool.tile([1, TB], F32, tag="rec",
                                             name=f"rec{T}t{ht}e{e}")
                            nc.vector.reciprocal(rec[:], po[e][64:65, :])
                            bc = bcpool.tile([64, TB], F32, tag="bc",
                                             name=f"bc{T}t{ht}e{e}")
                            nc.gpsimd.partition_broadcast(bc[:], rec[:])
                            with nc.allow_low_precision(reason="bf16 attn out"):
                                nc.vector.tensor_mul(
                                    attnT[ht][e * 64:e * 64 + 64, tcols],
                                    po[e][0:64, :], bc[:])

                def out_proj():
                    for go in range(4):
                        osb = opool.tile([128, 4 * EMBED], F16, tag="osb",
                                         name=f"osb{go}")
                        for ti in range(4):
                            tau = go * 4 + ti
                            ps = ps_s.tile([128, EMBED], F32, tag="s",
                                           name=f"ps_o{tau}")
                            for c in range(2):
                                nc.tensor.matmul(
                                    ps[:], attnT[c][:, tau * 128:(tau + 1) * 128],
                                    wosb[:, c * EMBED:(c + 1) * EMBED],
                                    start=(c == 0), stop=(c == 1))
                            with nc.allow_low_precision(reason="fp16 partial out"):
                                nc.any.tensor_copy(
                                    osb[:, ti * EMBED:(ti + 1) * EMBED], ps[:])
                        dst = bass.AP(
                            tensor=outp[:].tensor,
                            offset=go * 4 * 128 * EMBED,
                            ap=[[EMBED, 128], [128 * EMBED, 4], [1, EMBED]])
                        nc.sync.dma_start(dst, osb[:])

                for tb in range(NSTRIP):
                    proj_qk("q", tb, qTt)
                    proj_qk("k", tb, kTt)
                    proj_v(tb)
                    flash_strip(tb)
                out_proj()

            if reps > 1:
                with tc.For_i(0, reps, 1,
                              hint_engines=(mybir.EngineType.PE,
                                            mybir.EngineType.Activation,
                                            mybir.EngineType.DVE,
                                            mybir.EngineType.SP,
                                            mybir.EngineType.Pool)):
                    body()
            else:
                body()

    nc.compile()
    return nc


_PROGRAM_CACHE = {}


def get_program(causal: bool, use_mask: bool, has_bias: bool, reps: int = 1):
    key = (causal, use_mask, has_bias, reps)
    if key not in _PROGRAM_CACHE:
        _PROGRAM_CACHE[key] = _build_program(causal, use_mask, has_bias, reps)
    return _PROGRAM_CACHE[key]


def _prep_in_maps(query, key, value, key_padding_mask, attn_mask,
                  Wq, bq, Wk, bk, Wv, bv, Wo, bo, use_mask, has_bias):
    """Build the 8 per-core input dicts."""
    import ml_dtypes
    cq, sq, ck, sk = _xpos_tables()
    tabs = np.concatenate([cq, sq, ck, sk], axis=1).astype(np.float16)
    # tri01[s, t] = 1 where t >= s (keep), else 0  (probs layout [s, t])
    tri01 = np.where(np.arange(128)[None, :] >= np.arange(128)[:, None],
                     np.float32(1.0), np.float32(0.0)).astype(ml_dtypes.bfloat16)

    def xT(x):
        return np.ascontiguousarray(
            np.asarray(x, np.float32).T.astype(np.float16))

    xqTs = [xT(query[b]) for b in range(B)]
    xkTs = [xT(key[b]) for b in range(B)]
    xvTs = [xT(value[b]) for b in range(B)]

    masks = None
    if use_mask:
        am = np.asarray(attn_mask, np.float32)
        kp = np.asarray(key_padding_mask)
        masks = []
        for b in range(B):
            m = am.copy()
            if kp[b].any():
                m = m + np.where(kp[b], np.float32(-1e30),
                                 np.float32(0.0))[None, :]
            masks.append(np.ascontiguousarray(m.T.astype(np.float32)))

    Wq = np.asarray(Wq, np.float32); bq = np.asarray(bq, np.float32)
    Wk = np.asarray(Wk, np.float32); bk = np.asarray(bk, np.float32)
    Wv = np.asarray(Wv, np.float32); bv = np.asarray(bv, np.float32)
    Wo = np.asarray(Wo, np.float32)

    in_maps = []
    for core in range(N_CORES):
        b, hg = core // 2, core % 2
        hs = hg * HPC
        idx_p = np.concatenate(
            [hs * HD + hl * HD + _PERM64 for hl in range(HPC)])
        # sin-projection rows: within each head's 64-block, row r <- r XOR 32
        xor = (np.arange(256).reshape(HPC, HD)[:, (np.arange(HD) ^ 32)]
               ).reshape(-1)
        idx_s = idx_p[xor]
        idx_v = hs * HD + np.arange(HPC * HD)

        wpack = np.zeros((128, _WCOLS), np.float32)
        wsel = {"qc": (Wq, bq, idx_p), "qs": (Wq, bq, idx_s),
                "kc": (Wk, bk, idx_p), "ks": (Wk, bk, idx_s),
                "v": (Wv, bv, idx_v)}
        for i, nm in enumerate(_WNM):
            W, bias, idx = wsel[nm]
            WT = W[idx, :].T          # [512 embed-in, 256 sel dims]
            for c in range(4):
                wpack[:, i * 1024 + c * 256:(i * 1024) + (c + 1) * 256] = \
                    WT[c * 128:(c + 1) * 128, :]
            if has_bias:
                wpack[0, _WB_BASE + i * 256:_WB_BASE + (i + 1) * 256] = bias[idx]
        WoT = Wo[:, idx_v].T          # [256 attn dims, 512 embed-out]
        wpack[:, _WO_BASE:_WO_BASE + EMBED] = WoT[0:128, :]
        wpack[:, _WO_BASE + EMBED:_WO_BASE + 2 * EMBED] = WoT[128:256, :]

        m = {
            "xqT": xqTs[b], "xkT": xkTs[b], "xvT": xvTs[b],
            "wpack": wpack.astype(ml_dtypes.bfloat16),
            "tabs": tabs, "tri01": tri01,
        }
        if use_mask:
            m["maskT"] = masks[b]
        in_maps.append(m)
    return in_maps


def classify_mask(attn_mask, key_padding_mask):
    am = np.asarray(attn_mask, np.float32)
    kp = np.asarray(key_padding_mask)
    if not kp.any():
        causal = np.where(
            np.tril(np.ones((L, L), bool)), np.float32(0.0),
            np.float32(NEG)).astype(np.float32)
        if np.array_equal(am, causal):
            return True, False
        if not am.any():
            return False, False
    return False, True


def kernel(query, key, value, key_padding_mask, attn_mask,
           Wq, bq, Wk, bk, Wv, bv, Wo, bo):
    causal, use_mask = classify_mask(attn_mask, key_padding_mask)
    has_bias = bool(np.asarray(bq).any() or np.asarray(bk).any()
                    or np.asarray(bv).any())
    nc = get_program(causal, use_mask, has_bias, reps=1)
    in_maps = _prep_in_maps(query, key, value, key_padding_mask, attn_mask,
                            Wq, bq, Wk, bk, Wv, bv, Wo, bo, use_mask, has_bias)
    res = run_bass_kernel_spmd(nc, in_maps, list(range(N_CORES)))
    bo = np.asarray(bo, np.float32)
    out = np.empty((B, L, EMBED), np.float32)
    for b in range(B):
        out[b] = (res.results[2 * b]["outp"].astype(np.float32)
                  + res.results[2 * b + 1]["outp"].astype(np.float32)
                  + bo[None, :])
    return out


# revision 10
# speedup vs baseline: 1.4448x; 1.1245x over previous
"""Bass/Trainium2 kernel for nn_BerpXposMultiHeadedAttention (8-core SPMD).

Sharding: data-parallel over batch (4 batches x 2 cores) x tensor-parallel over
heads (4 heads per core).  Each core computes its 4 heads of flash-style xpos
attention for its batch plus the row-sharded partial out-projection; the host
sums the two partials per batch (the "all-reduce") and adds the output bias.

v2 design notes (HW-measured on trn2; 343us/rep baseline -> ~245us/rep):
- DMA count dominates fixed cost (~0.6us SP dispatch + ~0.6us shared HWDGE
  per DMA).  All projection weights ship in ONE packed [128, 6400] bf16
  tensor + wo in a second tile (separate tile so the rep-boundary reload
  does not serialize behind out_proj), xpos tables in ONE [128, 4L] fp16
  tensor, x in 4 whole-L chunks per tensor, output in 4 merged DMAs with a
  custom strided AP.  144 DMAs -> ~21.
- Softmax denominator broadcast without the DRAM round-trip: DVE reciprocal
  of PSUM row 64 -> [1, TB] SBUF, gpsimd partition_broadcast -> [64, TB],
  one DVE mul writes normalized attn.T into attnT, using a partition-shifted
  engine write for odd heads (verified legal on HW; stride-0 SBUF DMA src is
  not, which is why the old kernel bounced through DRAM).
- gpsimd (Pool) elementwise is a TRAP: moving the xpos adds / prob masking /
  memsets there regressed HW by 140us despite the cost model approving
  (software Q7 ops + per-op library reloads).  Only partition_broadcast
  (which has no other-engine equivalent) stays on Pool.
- Causal masking costs no PSUM ops: scores are exp'd unmasked (diag-block
  scores are bounded so exp cannot overflow) and the bf16 probs are
  multiplied by a 0/1 lower-tri tile on DVE (2-byte dtypes get DVE 2x mode).
- exp activations and P@V matmuls are narrowed to causally-live columns;
  no post-exp memsets.  The two heads sharing a qT/kT tile are processed in
  lockstep per s-block: their K=64 QK matmuls contract over partition row
  groups 0/64 and are issued back-to-back into different PSUM banks of one
  [128,1024] tile, giving one [128, 2x512] exp per s-block (80 activations
  instead of 124).
- QK^T stays fp32r x fp32r: HW-benched LDW+MM at K=64,N=512 is ~310ns for
  f32r vs ~380ns for fp16/bf16 (and exact), contrary to the cost model's
  213ns-for-everything.  Per-MM cost is LDWEIGHTS-exposed; 16-bit dtypes do
  not help matmul streams whose stationary changes every MM.
"""
import os
import sys

sys.path.insert(0, "/opt/trn_rl_repo")

_ABL = set(os.environ.get("ABL", "").split(","))

import contextlib

import numpy as np

import concourse.bacc as bacc
import concourse.bass as bass
import concourse.tile as tile
from concourse import mybir
from concourse.bass_utils import run_bass_kernel_spmd

# Problem constants (hardcoded per the task contract).
B = 4
L = 2048
EMBED = 512
HEADS = 8
HD = 64
SCALE_BASE = 512
NEG = -1e9
N_CORES = 8
HPC = 4           # heads per core
TB = 512          # t-block (strip) width
NS = L // 128     # 16 s-chunks
NSTRIP = L // TB  # 4 strips
VW = 328          # v_aug tile width (4 heads x 65 + 68 pad)

F32 = mybir.dt.float32
F32R = mybir.dt.float32r
F16 = mybir.dt.float16
BF16 = mybir.dt.bfloat16

# Deinterleave permutation of a 64-wide head dim: evens then odds.
_PERM64 = np.concatenate([np.arange(0, HD, 2), np.arange(1, HD, 2)])

# wpack column layout: 5 projections (qc,qs,kc,ks,v) x 4 chunks x 256 cols,
# then wo 2 chunks x 512 cols, then 5 bias rows x 256 cols (row 0 only).
_WNM = ["qc", "qs", "kc", "ks", "v"]
_WBASE = {nm: i * 1024 for i, nm in enumerate(_WNM)}
_WB_BASE = 5 * 1024
_WO_BASE = _WB_BASE + 5 * 256
_WCOLS = _WO_BASE + 2 * EMBED


def _xpos_tables():
    """Host-side xpos cos/sin tables in the permuted [d, t] layout.

    Returns (cq, sq, ck, sk), each [128, L] float32 (two heads' worth of rows,
    identical per head).  The 1/sqrt(HD) score scale is folded into the q pair.
    """
    d = HD
    base = ((np.arange(0, d, 2, dtype=np.float32) + np.float32(0.4 * d))
            / np.float32(1.4 * d)).astype(np.float32)                    # [32]
    min_pos = -(L // 2)
    power = (np.arange(min_pos, L + min_pos, dtype=np.float32)
             / np.float32(SCALE_BASE))                                   # [L]
    scale = (base[None, :] ** power[:, None]).astype(np.float32)         # [L, 32]
    half = d // 2
    inv_freq = (1.0 / (10000.0 ** (np.arange(half, dtype=np.float32) / half))
                ).astype(np.float32)
    sinusoid = np.arange(L, dtype=np.float32)[:, None] * inv_freq[None, :]
    sin = np.sin(sinusoid).astype(np.float32)
    cos = np.cos(sinusoid).astype(np.float32)

    def pack(cs, ss, fold):
        cs = (cs * fold).astype(np.float32)
        ss = (ss * fold).astype(np.float32)
        # permuted layout: rows 0:32 <- even orig dims, rows 32:64 <- odd.
        cos_p = np.concatenate([cs.T, cs.T], axis=0)      # [64, L]
        sin_p = np.concatenate([-ss.T, ss.T], axis=0)     # [64, L]
        return (np.concatenate([cos_p, cos_p], axis=0).astype(np.float32),
                np.concatenate([sin_p, sin_p], axis=0).astype(np.float32))

    inv_scale = (1.0 / scale).astype(np.float32)
    cq, sq = pack(cos * scale, sin * scale, np.float32(HD ** -0.5))
    ck, sk = pack(cos * inv_scale, sin * inv_scale, np.float32(1.0))
    return cq, sq, ck, sk


def _build_program(causal: bool, use_mask: bool, has_bias: bool, reps: int = 1):
    nc = bacc.Bacc("TRN2", target_bir_lowering=False, debug=False,
                   num_devices=N_CORES)

    # ---- DRAM I/O -------------------------------------------------------
    xqT = nc.dram_tensor("xqT", [EMBED, L], F16, kind="ExternalInput")
    xkT = nc.dram_tensor("xkT", [EMBED, L], F16, kind="ExternalInput")
    xvT = nc.dram_tensor("xvT", [EMBED, L], F16, kind="ExternalInput")
    wpackD = nc.dram_tensor("wpack", [128, _WCOLS], BF16, kind="ExternalInput")
    tabsD = nc.dram_tensor("tabs", [128, 4 * L], F16, kind="ExternalInput")
    triD = nc.dram_tensor("tri01", [128, 128], BF16, kind="ExternalInput")
    maskD = None
    if use_mask:
        maskD = nc.dram_tensor("maskT", [L, L], F32, kind="ExternalInput")
    outp = nc.dram_tensor("outp", [L, EMBED], F16, kind="ExternalOutput")

    xin = {"q": xqT, "k": xkT, "v": xvT}
    # tab column base per table name
    tb_base = {"cq": 0, "sq": L, "ck": 2 * L, "sk": 3 * L}

    with tile.TileContext(nc) as tc:
        with contextlib.ExitStack() as ctx:
            consts = ctx.enter_context(tc.tile_pool(name="consts", bufs=1))
            xpool = ctx.enter_context(tc.tile_pool(name="xpool", bufs=12))
            qkpool = ctx.enter_context(tc.tile_pool(name="qkpool", bufs=1))
            vpool = ctx.enter_context(tc.tile_pool(name="vpool", bufs=NS))
            tmp = ctx.enter_context(tc.tile_pool(name="tmp", bufs=4))
            ptpool = ctx.enter_context(tc.tile_pool(name="ptpool", bufs=10))
            npool = ctx.enter_context(tc.tile_pool(name="npool", bufs=4))
            bcpool = ctx.enter_context(tc.tile_pool(name="bcpool", bufs=4))
            opool = ctx.enter_context(tc.tile_pool(name="opool", bufs=2))
            mpool = None
            if use_mask:
                mpool = ctx.enter_context(tc.tile_pool(name="mpool", bufs=NS + 2))
            ps_s = ctx.enter_context(tc.tile_pool(name="ps_s", bufs=2, space="PSUM"))
            ps_pv = ctx.enter_context(tc.tile_pool(name="ps_pv", bufs=4, space="PSUM"))

            def body():
                # ---- constants / weights: 3 DMAs total ----
                wsb = consts.tile([128, _WO_BASE], BF16, tag="wsb", name="wsb")
                nc.sync.dma_start(wsb[:], wpackD[:, 0:_WO_BASE])
                wosb = consts.tile([128, 2 * EMBED], BF16, tag="wosb",
                                   name="wosb")
                nc.sync.dma_start(wosb[:], wpackD[:, _WO_BASE:_WCOLS])
                tabsb = consts.tile([128, 4 * L], F16, tag="tabs", name="tabs")
                nc.sync.dma_start(tabsb[:], tabsD[:])
                tri_sb = consts.tile([128, 128], BF16, tag="tri")
                if causal:
                    nc.sync.dma_start(tri_sb[:], triD[:])
                ones_sb = None
                if has_bias:
                    ones_sb = consts.tile([1, L], F16, tag="ones")
                    nc.vector.memset(ones_sb[:], 1.0)

                attnT = [consts.tile([128, L], BF16, tag=f"attnT{c}",
                                     name=f"attnT{c}") for c in range(2)]

                # ---- x loads: 12 DMAs of [128, L] ----
                xsb = {}
                for nm in ("q", "k", "v"):
                    chunks = []
                    for c in range(4):
                        t = xpool.tile([128, L], F16, tag="x",
                                       name=f"x{nm}{c}")
                        nc.sync.dma_start(t[:], xin[nm][c * 128:(c + 1) * 128, :])
                        chunks.append(t)
                    xsb[nm] = chunks

                qTt = [[None] * NSTRIP for _ in range(2)]  # [e][tb]
                kTt = [[None] * NSTRIP for _ in range(2)]
                vaug = [None] * NS

                def proj_qk(nm, tb, dst):
                    ts = slice(tb * TB, (tb + 1) * TB)
                    cb = tb_base[{"q": "cq", "k": "ck"}[nm]]
                    sb_ = tb_base[{"q": "sq", "k": "sk"}[nm]]
                    for e in range(2):
                        if "noproj" in _ABL:
                            ot = qkpool.tile([128, TB], F32R, tag=f"{nm}T{e}_{tb}",
                                             name=f"{nm}T{e}_{tb}")
                            dst[e][tb] = ot
                            continue
                        ps = ps_s.tile([128, 1024], F32, tag="s",
                                       name=f"ps_{nm}{e}_{tb}")
                        for half, wkey in ((0, nm + "c"), (1, nm + "s")):
                            wb = _WBASE[wkey]
                            for c in range(4):
                                nc.tensor.matmul(
                                    ps[:, half * TB:(half + 1) * TB],
                                    wsb[:, wb + c * 256 + e * 128:
                                        wb + c * 256 + e * 128 + 128],
                                    xsb[nm][c][:, ts],
                                    start=(c == 0),
                                    stop=(c == 3 and not has_bias))
                            if has_bias:
                                bi = _WNM.index(wkey)
                                nc.tensor.matmul(
                                    ps[:, half * TB:(half + 1) * TB],
                                    wsb[0:1, _WB_BASE + bi * 256 + e * 128:
                                        _WB_BASE + bi * 256 + e * 128 + 128],
                                    ones_sb[:, ts], start=False, stop=True)
                        t1 = tmp.tile([128, TB], F32, tag="t1",
                                      name=f"t1{nm}{e}{tb}")
                        nc.vector.tensor_mul(t1[:], ps[:, 0:TB],
                                             tabsb[:, cb + tb * TB:cb + (tb + 1) * TB])
                        t2 = tmp.tile([128, TB], F32, tag="t2",
                                      name=f"t2{nm}{e}{tb}")
                        nc.vector.tensor_mul(t2[:], ps[:, TB:1024],
                                             tabsb[:, sb_ + tb * TB:sb_ + (tb + 1) * TB])
                        ot = qkpool.tile([128, TB], F32R, tag=f"{nm}T{e}_{tb}",
                                         name=f"{nm}T{e}_{tb}")
                        with nc.allow_low_precision(reason="f32r is fp32 bits"):
                            nc.vector.tensor_add(ot[:], t1[:], t2[:])
                        dst[e][tb] = ot

                def proj_v(tb):
                    wb = _WBASE["v"]
                    for j in range(4):
                        si = tb * 4 + j
                        js = slice(si * 128, (si + 1) * 128)
                        ps = ps_pv.tile([128, 256], F32, tag="pv",
                                        name=f"ps_v{si}")
                        for c in range(4):
                            nc.tensor.matmul(ps[:], xsb["v"][c][:, js],
                                             wsb[:, wb + c * 256:wb + (c + 1) * 256],
                                             start=(c == 0),
                                             stop=(c == 3 and not has_bias))
                        if has_bias:
                            nc.tensor.matmul(
                                ps[:], ones_sb[:, js],
                                wsb[0:1, _WB_BASE + 4 * 256:_WB_BASE + 5 * 256],
                                start=False, stop=True)
                        va = vpool.tile([128, VW], BF16, tag="vaug",
                                        name=f"vaug{si}")
                        va3 = va[:, 0:HPC * 65].rearrange("p (h c) -> p h c", c=65)
                        nc.vector.tensor_copy(
                            va3[:, :, 0:64],
                            ps[:].rearrange("p (h d) -> p h d", d=64))
                        nc.vector.memset(va3[:, :, 64:65], 1.0)
                        nc.vector.memset(va[:, HPC * 65:VW], 0.0)
                        vaug[si] = va

                def flash_strip(T):
                    nsig = 4 * T + 4 if causal else NS
                    mtiles = None
                    if use_mask:
                        mtiles = []
                        for si in range(nsig):
                            mt = mpool.tile([128, TB], F32, tag="mask",
                                            name=f"m{T}_{si}")
                            nc.sync.dma_start(
                                mt[:], maskD[si * 128:(si + 1) * 128,
                                             T * TB:(T + 1) * TB])
                            mtiles.append(mt)
                    tcols = slice(T * TB, (T + 1) * TB)
                    for ht in range(2):
                        # Both heads of the pair advance together: their QK
                        # matmuls contract over disjoint 64-partition row
                        # groups (base 0 / 64) and are issued back-to-back so
                        # the PE array runs them concurrently (K=64 would
                        # otherwise idle half the rows).
                        po = [ps_pv.tile([128, TB], F32, tag="pv",
                                         name=f"po{T}t{ht}e{e}")
                              for e in range(2)]
                        for sig in range(nsig):
                            j = sig - 4 * T
                            jc = max(j, 0) if causal else 0
                            coff = 0
                            if causal and j >= 0:
                                coff = TB - max(TB - j * 128, 256)
                            ps2 = ps_s.tile([128, 1024], F32, tag="s",
                                            name=f"S{T}t{ht}s{sig}")
                            for e in range(2) if "noqk" not in _ABL else []:
                                hr = e * 64
                                nc.tensor.matmul(
                                    ps2[:, e * TB + coff:(e + 1) * TB],
                                    kTt[ht][sig // 4][hr:hr + 64,
                                                      (sig % 4) * 128:
                                                      (sig % 4 + 1) * 128],
                                    qTt[ht][T][hr:hr + 64, coff:TB],
                                    start=True, stop=True)
                            if use_mask:
                                for e in range(2):
                                    sl = slice(e * TB, (e + 1) * TB)
                                    nc.vector.tensor_add(ps2[:, sl], ps2[:, sl],
                                                         mtiles[sig][:])
                            pt = ptpool.tile([128, 1024], BF16, tag="pt",
                                             name=f"P{T}t{ht}s{sig}")
                            if "noexp" in _ABL:
                                pass
                            elif jc > 0:
                                pt3 = pt.rearrange("p (h t) -> p h t", t=TB)
                                ps3 = ps2.rearrange("p (h t) -> p h t", t=TB)
                                nc.scalar.activation(
                                    pt3[:, :, jc * 128:TB],
                                    ps3[:, :, jc * 128:TB],
                                    mybir.ActivationFunctionType.Exp)
                            else:
                                nc.scalar.activation(
                                    pt[:], ps2[:],
                                    mybir.ActivationFunctionType.Exp)
                            if causal and 0 <= j <= 3:
                                for e in range(2):
                                    sl = slice(e * TB + j * 128,
                                               e * TB + (j + 1) * 128)
                                    with nc.allow_low_precision(
                                            reason="bf16 prob masking"):
                                        nc.vector.tensor_mul(
                                            pt[:, sl], pt[:, sl], tri_sb[:])
                            for e in range(2) if "nopv" not in _ABL else []:
                                nc.tensor.matmul(
                                    po[e][:, jc * 128:TB],
                                    vaug[sig][:, (2 * ht + e) * 65:
                                              (2 * ht + e) * 65 + 128],
                                    pt[:, e * TB + jc * 128:(e + 1) * TB],
                                    start=(sig == 0), stop=(sig == nsig - 1))
                        # normalization: reciprocal of the denominator row,
                        # partition-broadcast it (Pool), one DVE mul writes
                        # normalized attn.T into attnT (partition-shifted for
                        # odd heads).
                        for e in range(2) if "nonorm" not in _ABL else []:
                            rec = npool.tile([1, TB], F32, tag="rec",
                                             name=f"rec{T}t{ht}e{e}")
                            nc.vector.reciprocal(rec[:], po[e][64:65, :])
                            bc = bcpool.tile([64, TB], F32, tag="bc",
                                             name=f"bc{T}t{ht}e{e}")
                            nc.gpsimd.partition_broadcast(bc[:], rec[:])
                            with nc.allow_low_precision(reason="bf16 attn out"):
                                nc.vector.tensor_mul(
                                    attnT[ht][e * 64:e * 64 + 64, tcols],
                                    po[e][0:64, :], bc[:])

                def out_proj(go):
                    if True:
                        osb = opool.tile([128, 4 * EMBED], F16, tag="osb",
                                         name=f"osb{go}")
                        for ti in range(4):
                            tau = go * 4 + ti
                            ps = ps_s.tile([128, EMBED], F32, tag="s",
                                           name=f"ps_o{tau}")
                            for c in range(2):
                                nc.tensor.matmul(
                                    ps[:], attnT[c][:, tau * 128:(tau + 1) * 128],
                                    wosb[:, c * EMBED:(c + 1) * EMBED],
                                    start=(c == 0), stop=(c == 1))
                            with nc.allow_low_precision(reason="fp16 partial out"):
                                nc.any.tensor_copy(
                                    osb[:, ti * EMBED:(ti + 1) * EMBED], ps[:])
                        dst = bass.AP(
                            tensor=outp[:].tensor,
                            offset=go * 4 * 128 * EMBED,
                            ap=[[EMBED, 128], [128 * EMBED, 4], [1, EMBED]])
                        nc.sync.dma_start(dst, osb[:])

                for tb in range(NSTRIP):
                    proj_qk("q", tb, qTt)
                    proj_qk("k", tb, kTt)
                    proj_v(tb)
                    flash_strip(tb)
                    # out-proj group tb only needs attnT columns written by
                    # strip tb: overlap it with the next strip instead of
                    # serializing all 16 taus into the tail.
                    out_proj(tb)

            if reps > 1:
                with tc.For_i(0, reps, 1,
                              hint_engines=(mybir.EngineType.PE,
                                            mybir.EngineType.Activation,
                                            mybir.EngineType.DVE,
                                            mybir.EngineType.SP,
                                            mybir.EngineType.Pool)):
                    body()
            else:
                body()

    nc.compile()
    return nc


_PROGRAM_CACHE = {}


def get_program(causal: bool, use_mask: bool, has_bias: bool, reps: int = 1):
    key = (causal, use_mask, has_bias, reps)
    if key not in _PROGRAM_CACHE:
        _PROGRAM_CACHE[key] = _build_program(causal, use_mask, has_bias, reps)
    return _PROGRAM_CACHE[key]


def _prep_in_maps(query, key, value, key_padding_mask, attn_mask,
                  Wq, bq, Wk, bk, Wv, bv, Wo, bo, use_mask, has_bias):
    """Build the 8 per-core input dicts."""
    import ml_dtypes
    cq, sq, ck, sk = _xpos_tables()
    tabs = np.concatenate([cq, sq, ck, sk], axis=1).astype(np.float16)
    # tri01[s, t] = 1 where t >= s (keep), else 0  (probs layout [s, t])
    tri01 = np.where(np.arange(128)[None, :] >= np.arange(128)[:, None],
                     np.float32(1.0), np.float32(0.0)).astype(ml_dtypes.bfloat16)

    def xT(x):
        return np.ascontiguousarray(
            np.asarray(x, np.float32).T.astype(np.float16))

    xqTs = [xT(query[b]) for b in range(B)]
    xkTs = [xT(key[b]) for b in range(B)]
    xvTs = [xT(value[b]) for b in range(B)]

    masks = None
    if use_mask:
        am = np.asarray(attn_mask, np.float32)
        kp = np.asarray(key_padding_mask)
        masks = []
        for b in range(B):
            m = am.copy()
            if kp[b].any():
                m = m + np.where(kp[b], np.float32(-1e30),
                                 np.float32(0.0))[None, :]
            masks.append(np.ascontiguousarray(m.T.astype(np.float32)))

    Wq = np.asarray(Wq, np.float32); bq = np.asarray(bq, np.float32)
    Wk = np.asarray(Wk, np.float32); bk = np.asarray(bk, np.float32)
    Wv = np.asarray(Wv, np.float32); bv = np.asarray(bv, np.float32)
    Wo = np.asarray(Wo, np.float32)

    in_maps = []
    for core in range(N_CORES):
        b, hg = core // 2, core % 2
        hs = hg * HPC
        idx_p = np.concatenate(
            [hs * HD + hl * HD + _PERM64 for hl in range(HPC)])
        # sin-projection rows: within each head's 64-block, row r <- r XOR 32
        xor = (np.arange(256).reshape(HPC, HD)[:, (np.arange(HD) ^ 32)]
               ).reshape(-1)
        idx_s = idx_p[xor]
        idx_v = hs * HD + np.arange(HPC * HD)

        wpack = np.zeros((128, _WCOLS), np.float32)
        wsel = {"qc": (Wq, bq, idx_p), "qs": (Wq, bq, idx_s),
                "kc": (Wk, bk, idx_p), "ks": (Wk, bk, idx_s),
                "v": (Wv, bv, idx_v)}
        for i, nm in enumerate(_WNM):
            W, bias, idx = wsel[nm]
            WT = W[idx, :].T          # [512 embed-in, 256 sel dims]
            for c in range(4):
                wpack[:, i * 1024 + c * 256:(i * 1024) + (c + 1) * 256] = \
                    WT[c * 128:(c + 1) * 128, :]
            if has_bias:
                wpack[0, _WB_BASE + i * 256:_WB_BASE + (i + 1) * 256] = bias[idx]
        WoT = Wo[:, idx_v].T          # [256 attn dims, 512 embed-out]
        wpack[:, _WO_BASE:_WO_BASE + EMBED] = WoT[0:128, :]
        wpack[:, _WO_BASE + EMBED:_WO_BASE + 2 * EMBED] = WoT[128:256, :]

        m = {
            "xqT": xqTs[b], "xkT": xkTs[b], "xvT": xvTs[b],
            "wpack": wpack.astype(ml_dtypes.bfloat16),
            "tabs": tabs, "tri01": tri01,
        }
        if use_mask:
            m["maskT"] = masks[b]
        in_maps.append(m)
    return in_maps


def classify_mask(attn_mask, key_padding_mask):
    am = np.asarray(attn_mask, np.float32)
    kp = np.asarray(key_padding_mask)
    if not kp.any():
        causal = np.where(
            np.tril(np.ones((L, L), bool)), np.float32(0.0),
            np.float32(NEG)).astype(np.float32)
        if np.array_equal(am, causal):
            return True, False
        if not am.any():
            return False, False
    return False, True


def kernel(query, key, value, key_padding_mask, attn_mask,
           Wq, bq, Wk, bk, Wv, bv, Wo, bo):
    causal, use_mask = classify_mask(attn_mask, key_padding_mask)
    has_bias = bool(np.asarray(bq).any() or np.asarray(bk).any()
                    or np.asarray(bv).any())
    nc = get_program(causal, use_mask, has_bias, reps=1)
    in_maps = _prep_in_maps(query, key, value, key_padding_mask, attn_mask,
                            Wq, bq, Wk, bk, Wv, bv, Wo, bo, use_mask, has_bias)
    res = run_bass_kernel_spmd(nc, in_maps, list(range(N_CORES)))
    bo = np.asarray(bo, np.float32)
    out = np.empty((B, L, EMBED), np.float32)
    for b in range(B):
        out[b] = (res.results[2 * b]["outp"].astype(np.float32)
                  + res.results[2 * b + 1]["outp"].astype(np.float32)
                  + bo[None, :])
    return out
